# revision 60
# baseline (speedup 1.0000x reference)
"""MoE routing kernel (nn_JSMLP): per-row expert-indexed 3-layer MLP.

  out[n] = Wl[i] @ tanh(W2[i] @ tanh(W1[i] @ x[n] + b1[i]) + b2[i]) + bl[i],  i = ind[n]

Sharding (hardcoded): expert-parallel across 8 cores, load-balanced by
count-sorted round-robin so a single SPMD program fits all cores:
experts are sorted by row count (desc); rank r goes to core r%8, slot r//8.
Slot s then has the same capacity cap[s] = roundup(count of rank 8s, 2) on
every core, so per-slot column spans are compile-time constants while padding
stays ~2% (vs ~50% for a uniform max-count capacity). Slots are grouped into
blocks of <=4 (span <=512 = one PSUM bank), executed in a pyramid order
(small block first for pipeline fill + PE p-state warm-up, big blocks
mid-stream, one 2-slot block last so the drain after the final weight
DMA is short).

Numerics: W1 (with b1 row), W2 and the first half of Wl are stored as
float8_e3m4 scaled by 64 (values land in e3m4's normal range; 4 mantissa
bits ~ 1.2% rms/elem); the 1/64 descale rides the tanh activations and the
output copy (tensor_scalar_mul) for free. Wl's second half is bf16 (also
x64, exact). x, h, biases stay bf16; PSUM accumulates fp32. Measured
end-to-end rel err 1.85e-2 vs the fp32 reference (gate: 2e-2).

Per core, per block b:
  L1: H1T[256, S_b] = W1augT.T @ [x;1]T   (bias via ones-row, K=65, e3m4 x64)
  L2: H2T[256, S_b] = W2T.T @ tanh(H1T/64)  (K=256 in 2 chunks; b2 added last
      via a tiny block-diag ones matmul in bf16, also x64)
  L3: outT[2x64, PS_b] = WlT.T @ tanh(H2T/64)  (2 experts per 128-partition
      tile; bl added last via pair-diag ones; plain bf16)
Within each PSUM accumulation group only the first matmul has start=True
(start marks the whole 2KB zero-region pending-zero, so per-region starts
would wipe neighbors); bias matmuls go last because the first matmul after a
pipeline stall runs at the mid p-state and should be a narrow chunk, not the
full-span bias. The block loop is software-pipelined as [L2(i), L3(i-1),
L1(i+2)] per iteration with one shared L1/L2 PSUM tile per block (3-deep
ring + two 1-bank L3 tiles = all 8 banks) so the in-order PE queue never
stalls on ACT between layers.

DMA plan (the shared DMA engine pool is the roofline at ~360 B/ns, and the
serial HWDGE costs ~630ns per DMA): x and W1 are packed per-block into one
byte-merged e3m4 tensor loaded in 3 slices; W2+Wl are packed per-block into
another (Wl columns are read through bitcast(bf16) APs), W2 loading ahead of
Wl for the first blocks and as one slab later; per-2-block stores ride the
idle GPSIMD/SWDGE path (no HWDGE slot), with the final store on SP. All
loads land in resident SBUF tiles (no ring reuse) so the load queue never
blocks on consumers.
"""

import numpy as np
import ml_dtypes

N, IN_DIM, H1, H2, LIN, NEXP = 16384, 64, 256, 256, 64, 256
NCORES = 8
SLOTS = NEXP // NCORES  # 32 experts per core

BF16 = ml_dtypes.bfloat16
E3M4 = ml_dtypes.float8_e3m4
WSCALE = 64.0

_cache = {}


def _geometry(caps):
    """Block/pair geometry shared by program builder and host prep.

    caps: per-slot capacities (len 32, multiples of 4, may be 0).
    Returns dict with blocks (list of slot-index lists), per-block slot
    offsets, block x-offsets, pair layout and output offsets.
    """
    blocks = []
    cur, cur_sum = [], 0
    for s in range(SLOTS):
        c = caps[s]
        if c == 0:
            continue
        if len(cur) == 5 or (cur_sum + c > 512 and cur):
            blocks.append(cur)
            cur, cur_sum = [], 0
        cur.append(s)
        cur_sum += c
    if cur:
        blocks.append(cur)
    # one tiny 2-slot trailing block shortens the drain after the last
    # weight DMA with a single end-chain round of engine hops
    if len(blocks[-1]) > 2:
        s2 = blocks[-1].pop()
        s1 = blocks[-1].pop()
        blocks.append([s1, s2])
    # pyramid execution order: a small block first (quick pipeline fill and
    # PE p-state warm-up), big blocks mid-stream, tiny block last (short
    # drain after the final weight DMA)
    if len(blocks) > 3:
        tail, normals = blocks[-1:], blocks[:-1]
        ns = sorted(normals, key=lambda bl: sum(caps[s] for s in bl))
        blocks = ns[0::2] + ns[1::2][::-1] + tail

    # equalize capacities within each L3 pair (~1.5% extra padding) so the
    # packed output tile has no gap columns — every PSUM element is written
    # by a chunk matmul before the trailing bias accumulation
    caps = list(caps)
    for bl in blocks:
        for i in range(0, len(bl) - 1, 2):
            m = max(caps[bl[i]], caps[bl[i + 1]])
            caps[bl[i]] = caps[bl[i + 1]] = m

    g = {"blocks": blocks, "caps": caps, "xoff": [], "S": [], "XO": [],
         "pairs": [], "poff": [], "PS": [], "OO": []}
    xo_total, oo_total = 0, 0
    for bl in blocks:
        offs, acc = [], 0
        for s in bl:
            offs.append(acc)
            acc += caps[s]
        assert acc <= 512, f"block span {acc} exceeds a PSUM bank"
        g["xoff"].append(offs)
        g["S"].append(acc)
        g["XO"].append(xo_total)
        xo_total += acc
        prs = [(bl[i], bl[i + 1] if i + 1 < len(bl) else None)
               for i in range(0, len(bl), 2)]
        poffs, pacc = [], 0
        for a, b in prs:
            poffs.append(pacc)
            pacc += max(caps[a], caps[b] if b is not None else 0)
        g["pairs"].append(prs)
        g["poff"].append(poffs)
        g["PS"].append(pacc)
        g["OO"].append(oo_total)
        oo_total += pacc
    g["TOT"] = xo_total
    g["TOT2"] = oo_total
    # weight tensors are laid out in execution order: slot s lives at
    # column-group scol[s], so per-block loads are contiguous slices
    scol, idx = {}, 0
    for bl in blocks:
        for s in bl:
            scol[s] = idx
            idx += 1
    g["scol"] = scol
    return g


def _build_program(caps):
    import concourse.bass as bass
    import concourse.tile as tile
    from concourse import bacc, mybir

    g = _geometry(caps)
    caps = g["caps"]
    blocks, S, XO, xoff = g["blocks"], g["S"], g["XO"], g["xoff"]
    pairs, poff, PS, OO = g["pairs"], g["poff"], g["PS"], g["OO"]
    NB = len(blocks)
    TOT, TOT2 = g["TOT"], g["TOT2"]
    scol = g["scol"]

    f32 = mybir.dt.float32
    bf16 = mybir.dt.bfloat16
    e3 = mybir.dt.float8e3
    Tanh = mybir.ActivationFunctionType.Tanh

    # ct columns: [w2c: NB*256 | wlc: NB*128 (rows 0:2) | bdt: TOT | bdl: TOT2]
    O_WLC = NB * 256
    O_BDT = O_WLC + NB * 128
    O_BDL = O_BDT + TOT
    CTW = O_BDL + TOT2

    nc = bacc.Bacc("TRN2", target_bir_lowering=False, debug=False,
                   num_devices=NCORES)

    # xw: per exec block [xg bytes (2*S_b) | w1 (nslots*256)], all e3m4 bytes
    XB = []
    off = 0
    for b in range(NB):
        XB.append(off)
        off += 2 * S[b] + 256 * len(blocks[b])
    XWW = off
    xw_d = nc.dram_tensor("xw", [65, XWW], e3, kind="ExternalInput")
    # wb: per exec block [w2: nslots*512 | wl(bf16 bytes): nslots*128],
    # so a block's W2 can load separately from (ahead of) its Wl
    WBO = []
    wboff = 0
    for b in range(NB):
        WBO.append(wboff)
        wboff += 704 * len(blocks[b])
    WBW = wboff
    wb_d = nc.dram_tensor("wb", [128, WBW], e3, kind="ExternalInput")
    ct_d = nc.dram_tensor("ct", [8, CTW], bf16, kind="ExternalInput")
    out_d = nc.dram_tensor("out", [128, TOT2], bf16, kind="ExternalOutput")

    # block -> first/last weight-column groups (exec-order layout)
    def slot_range(b0, b1):
        lo = scol[blocks[b0][0]]
        hi = scol[blocks[b1][-1]] + 1
        return lo, hi

    with tile.TileContext(nc) as tc:
        with (
            tc.tile_pool(name="stat", bufs=1) as spool,
            tc.tile_pool(name="acts", bufs=NB) as hpool,
            tc.tile_pool(name="ph", bufs=3, space=bass.MemorySpace.PSUM) as php,
            tc.tile_pool(name="po", bufs=2, space=bass.MemorySpace.PSUM) as pop,
        ):
            ct = spool.tile([8, CTW], bf16, tag="ct")
            xwt = spool.tile([65, XWW], e3, tag="xw")
            wbt = spool.tile([128, WBW], e3, tag="wb")
            ostage = spool.tile([128, TOT2], bf16, tag="out")

            # ---- load schedule -------------------------------------------
            # xw (x + W1, block-interleaved) in two slices; W2 per block in
            # execution order. Early blocks' Wl defers behind their W2 (L3
            # lags L2 by a block), later blocks load as one merged slab.
            def loadslice(lo, hi):
                nc.sync.dma_start(wbt[:, lo:hi], wb_d.ap()[:, lo:hi])

            def w2_rng(b):
                return WBO[b], WBO[b] + 512 * len(blocks[b])

            def wl_rng(b):
                return WBO[b] + 512 * len(blocks[b]), \
                    WBO[b] + 704 * len(blocks[b])

            NSPLIT = min(2, NB)
            b3 = min(2, NB - 1)
            b6 = min(5, NB - 1)
            xs1 = XB[b3] if 0 < b3 < NB else XWW
            xs2 = XB[b6] if b3 < b6 < NB else XWW
            nc.sync.dma_start(xwt[:, 0:xs1], xw_d.ap()[:, 0:xs1])
            loadslice(*w2_rng(0))
            nc.sync.dma_start(ct[:], ct_d.ap())
            if NB > 1:
                loadslice(*w2_rng(1))
            if xs1 < xs2:
                nc.sync.dma_start(xwt[:, xs1:xs2], xw_d.ap()[:, xs1:xs2])
            loadslice(*wl_rng(0))
            for b in range(2, NSPLIT):
                loadslice(*w2_rng(b))
                loadslice(*wl_rng(b - 1))
            if xs2 < XWW:
                nc.sync.dma_start(xwt[:, xs2:], xw_d.ap()[:, xs2:])
            if NSPLIT >= 2:
                loadslice(*wl_rng(NSPLIT - 1))
            for b in range(NSPLIT, NB):
                loadslice(WBO[b], WBO[b] + 704 * len(blocks[b]))

            # store after every 2nd block (idle Pool/SWDGE path, no HWDGE
            # slot); the final store rides SP whose HWDGE is free by then and
            # ~400ns faster than SWDGE on the drain-critical path
            store_after = {}
            lo_b = 0
            for b in range(1, NB, 2):
                store_after[b] = (lo_b, b)
                lo_b = b + 1
            if lo_b < NB:
                store_after[NB - 1] = (lo_b, NB - 1)

            def pool_activation(out, in_, func, scale=1.0):
                # InstActivation on the (otherwise idle) GPSIMD engine:
                # same semantics as nc.scalar.activation, modeled by the
                # cost model at the generic GPSIMD software efficiency
                eng = nc.gpsimd
                bias = eng.bass.const_aps.scalar_like(0.0, in_)
                ins = [eng.lower_ap(in_), eng.lower_ap(bias)]
                for arg in (scale, 0.0):
                    ins.append(mybir.ImmediateValue(dtype=f32, value=arg))
                return eng.add_instruction(mybir.InstActivation(
                    name=eng.bass.get_next_instruction_name(),
                    func=func, ins=ins, outs=[eng.lower_ap(out)]))

            # ---- software-pipelined compute ------------------------------
            # One PSUM tile per block serves both L1 and L2 (L2's start=True
            # rezeroes it after tanh1 has read it), so a 3-deep ring plus two
            # 1-bank L3 tiles fits the 8 PSUM banks without stalling.
            def emit_l1(b):
                bslots, Sb, xob = blocks[b], S[b], xoff[b]
                ph1 = php.tile([128, 1024], f32, tag="ph", name=f"ph_{b}")
                for t in range(2):
                    for i, s in enumerate(bslots):
                        c = caps[s]
                        w1o = XB[b] + 2 * Sb + i * 256 + t * 128
                        nc.tensor.matmul(
                            ph1[:, t * 512 + xob[i]: t * 512 + xob[i] + c],
                            xwt[:, w1o: w1o + 128],
                            xwt[:, XB[b] + 2 * xob[i]: XB[b] + 2 * (xob[i] + c)].bitcast(bf16),
                        )
                h1 = hpool.tile([128, 2 * Sb], bf16, tag="h1",
                                padded_shape=[128, 2 * max(S)], name=f"h1_{b}")
                nc.scalar.activation(
                    h1[:].rearrange("p (t s) -> p t s", t=2),
                    ph1[:].rearrange("p (t s) -> p t s", t=2)[:, :, 0:Sb],
                    Tanh, scale=1.0 / WSCALE,
                )
                return h1, ph1

            def emit_l2(b, h1, ph2):
                # bias matmul last: the first matmul after a pipeline stall
                # runs at the mid p-state, so it should be a narrow chunk
                # (F=c), not the full-span bias (F=S)
                bslots, Sb, xob = blocks[b], S[b], xoff[b]
                for t in range(2):
                    for i, s in enumerate(bslots):
                        c = caps[s]
                        nc.tensor.matmul(
                            ph2[:, t * 512 + xob[i]: t * 512 + xob[i] + c],
                            wbt[:, WBO[b] + i * 512 + t * 128: WBO[b] + i * 512 + (t + 1) * 128],
                            h1[:, xob[i]: xob[i] + c],
                            start=i == 0, stop=False, skip_group_check=True,
                        )
                        nc.tensor.matmul(
                            ph2[:, t * 512 + xob[i]: t * 512 + xob[i] + c],
                            wbt[:, WBO[b] + i * 512 + 256 + t * 128: WBO[b] + i * 512 + 256 + (t + 1) * 128],
                            h1[:, Sb + xob[i]: Sb + xob[i] + c],
                            start=False, stop=False, skip_group_check=True,
                        )
                    nc.tensor.matmul(
                        ph2[:, t * 512: t * 512 + Sb],
                        ct[:, (b * 2 + t) * 128: (b * 2 + t + 1) * 128],
                        ct[:, O_BDT + XO[b]: O_BDT + XO[b] + Sb],
                        start=False, stop=True, skip_group_check=True,
                    )
                h2 = hpool.tile([128, 2 * Sb], bf16, tag="h2",
                                padded_shape=[128, 2 * max(S)], name=f"h2_{b}")
                nc.scalar.activation(
                    h2[:].rearrange("p (t s) -> p t s", t=2),
                    ph2[:].rearrange("p (t s) -> p t s", t=2)[:, :, 0:Sb],
                    Tanh, scale=1.0 / WSCALE,
                )
                return h2

            def emit_l3(b, h2):
                Sb, xob = S[b], xoff[b]
                po = pop.tile([128, PS[b]], f32, tag="po",
                              padded_shape=[128, max(PS)], name=f"po_{b}")
                started = {0: False, 1: False}
                for ci, (sa, sb_) in enumerate(pairs[b]):
                    for h, s in enumerate((sa, sb_)):
                        if s is None:
                            continue
                        c = caps[s]
                        xo = xob[2 * ci + h]
                        wlo = WBO[b] + 512 * len(blocks[b]) + (2 * ci + h) * 192
                        nc.tensor.matmul(
                            po[h * 64:(h + 1) * 64,
                               poff[b][ci]: poff[b][ci] + c],
                            wbt[:, wlo: wlo + 64],
                            h2[:, xo: xo + c],
                            start=not started[h], stop=False,
                            skip_group_check=True,
                        )
                        started[h] = True
                        nc.tensor.matmul(
                            po[h * 64:(h + 1) * 64,
                               poff[b][ci]: poff[b][ci] + c],
                            wbt[:, wlo + 64: wlo + 192].bitcast(bf16),
                            h2[:, Sb + xo: Sb + xo + c],
                            start=False, stop=False, skip_group_check=True,
                        )
                # pair-gap columns may accumulate onto stale PSUM here; those
                # columns are never unscattered
                for h in range(2):
                    nc.tensor.matmul(
                        po[h * 64:(h + 1) * 64, 0:PS[b]],
                        ct[0:4, O_WLC + b * 128 + h * 64: O_WLC + b * 128 + (h + 1) * 64],
                        ct[0:4, O_BDL + OO[b]: O_BDL + OO[b] + PS[b]],
                        start=False, stop=h == 1, skip_group_check=True,
                    )
                nc.vector.tensor_scalar_mul(ostage[:, OO[b]: OO[b] + PS[b]],
                                            po[:, 0:PS[b]], 1.0 / WSCALE)
                if b in store_after:
                    b0, b1 = store_after[b]
                    eng = nc.sync if b1 == NB - 1 else nc.gpsimd
                    eng.dma_start(
                        out_d.ap()[:, OO[b0]: OO[b1] + PS[b1]],
                        ostage[:, OO[b0]: OO[b1] + PS[b1]],
                    )


            # software pipeline, PE order per iteration: L2(i), L3(i-1),
            # L1(i+2). The lookahead keeps every PSUM-ring WAR dependency one
            # full iteration old by the time it's needed, so the in-order PE
            # queue never stalls on ACT.
            h1s, h2s, phs = {}, {}, {}
            LOOK = min(2, NB)
            for i in range(LOOK):
                h1s[i], phs[i] = emit_l1(i)
            for i in range(NB):
                h2s[i] = emit_l2(i, h1s.pop(i), phs.pop(i))
                if i >= 1:
                    emit_l3(i - 1, h2s.pop(i - 1))
                if i + LOOK < NB:
                    h1s[i + LOOK], phs[i + LOOK] = emit_l1(i + LOOK)
            emit_l3(NB - 1, h2s.pop(NB - 1))

    nc.compile()
    return nc


def _plan(ind):
    counts = np.bincount(ind, minlength=NEXP)
    perm = np.argsort(-counts, kind="stable")
    caps = []
    for s in range(SLOTS):
        c = int(counts[perm[8 * s]])
        caps.append(0 if c == 0 else int(np.ceil(c / 2)) * 2)
    return counts, perm, caps


def _prep_inputs(x, ind, W1, b1, W2, b2, Wl, bl, perm, caps, g):
    """Build per-core arrays for the count-sorted round-robin layout."""
    blocks, S, XO, xoff = g["blocks"], g["S"], g["XO"], g["xoff"]
    pairs, poff, OO = g["pairs"], g["poff"], g["OO"]
    scol = g["scol"]
    caps = g["caps"]
    NB = len(blocks)
    TOT, TOT2 = g["TOT"], g["TOT2"]
    O_WLC = NB * 256
    O_BDT = O_WLC + NB * 128
    O_BDL = O_BDT + TOT
    CTW = O_BDL + TOT2

    order = np.argsort(ind, kind="stable")
    offs = np.zeros(NEXP + 1, np.int64)
    np.cumsum(np.bincount(ind, minlength=NEXP), out=offs[1:])
    rows = [order[offs[e]:offs[e + 1]] for e in range(NEXP)]

    # scaled transposed weights, shared across cores
    w1aug = np.concatenate([W1, b1[:, :, None]], axis=2)       # [E, 256, 65]
    w1q = (w1aug * WSCALE).astype(E3M4)                        # e3m4 x64
    w2q = (W2 * WSCALE).astype(E3M4)                           # [E, 256, 256]
    wlaq = (Wl[:, :, 0:128] * WSCALE).astype(E3M4)             # [E, 64, 128]
    wlbq = (Wl[:, :, 128:256] * WSCALE).astype(BF16)           # exact pow2
    b2q = (b2 * WSCALE).astype(np.float32)
    xb = x.astype(BF16)

    # xw/wb section offsets, mirroring the program builder
    XB, WBO = [], []
    off, wboff = 0, 0
    for b in range(NB):
        XB.append(off)
        off += 2 * S[b] + 256 * len(blocks[b])
        WBO.append(wboff)
        wboff += 704 * len(blocks[b])
    XWW, WBW = off, wboff

    in_maps = []
    for k in range(NCORES):
        xw = np.zeros((65, XWW), np.uint8)
        wb = np.zeros((128, WBW), np.uint8)
        ct = np.zeros((8, CTW), np.float32)
        ones = np.ones(1, BF16).view(np.uint8)
        for b in range(NB):
            Sb = S[b]
            for i, s in enumerate(blocks[b]):
                if caps[s] == 0:
                    continue
                e = perm[8 * s + k]
                r = rows[e]
                xo = XB[b] + 2 * xoff[b][i]
                xw[0:64, xo: xo + 2 * len(r)] = \
                    np.ascontiguousarray(xb[r].T).view(np.uint8)
                xw[64, xo: xo + 2 * caps[s]] = np.tile(ones, caps[s])
                w1o = XB[b] + 2 * Sb + i * 256
                xw[:, w1o: w1o + 256] = \
                    np.ascontiguousarray(w1q[e].T).view(np.uint8)
                w2o = WBO[b] + i * 512
                wb[:, w2o: w2o + 256] = \
                    np.ascontiguousarray(w2q[e, :, 0:128].T).view(np.uint8)
                wb[:, w2o + 256: w2o + 512] = \
                    np.ascontiguousarray(w2q[e, :, 128:256].T).view(np.uint8)
                wlo = WBO[b] + 512 * len(blocks[b]) + i * 192
                wb[:, wlo: wlo + 64] = \
                    np.ascontiguousarray(wlaq[e].T).view(np.uint8)
                wb[:, wlo + 64: wlo + 192] = \
                    np.ascontiguousarray(wlbq[e].T).view(np.uint8)
                ct[i, b * 256:(b + 1) * 256] = b2q[e]
                ct[i, O_BDT + XO[b] + xoff[b][i]:
                    O_BDT + XO[b] + xoff[b][i] + caps[s]] = 1.0
            for ci, (sa, sb_) in enumerate(pairs[b]):
                pc = OO[b] + poff[b][ci]
                w = max(caps[sa], caps[sb_] if sb_ is not None else 0)
                ct[ci, O_BDL + pc: O_BDL + pc + w] = 1.0
                for h, s in enumerate((sa, sb_)):
                    if s is None or caps[s] == 0:
                        continue
                    e = perm[8 * s + k]
                    ct[ci, O_WLC + b * 128 + h * 64: O_WLC + b * 128 + (h + 1) * 64] = bl[e] * WSCALE
        in_maps.append({
            "xw": xw.view(E3M4),
            "wb": wb.view(E3M4),
            "ct": ct.astype(BF16),
        })
    return in_maps, rows


def _unscatter(results, rows, perm, caps, g):
    blocks, xoff, poff, OO, pairs = g["blocks"], g["xoff"], g["poff"], g["OO"], g["pairs"]
    out = np.empty((N, LIN), np.float32)
    for k in range(NCORES):
        arr = np.asarray(results[k]["out"], np.float32)
        for b in range(len(blocks)):
            for ci, (sa, sb_) in enumerate(pairs[b]):
                for h, s in enumerate((sa, sb_)):
                    if s is None or caps[s] == 0:
                        continue
                    e = perm[8 * s + k]
                    r = rows[e]
                    col = OO[b] + poff[b][ci]
                    out[r, :] = arr[h * 64:(h + 1) * 64, col: col + len(r)].T
    return out


def kernel(x, ind, W1, b1, W2, b2, Wl, bl):
    from concourse.bass_utils import run_bass_kernel_spmd

    x = np.asarray(x, np.float32)
    ind = np.asarray(ind).astype(np.int64)
    W1 = np.asarray(W1, np.float32); b1 = np.asarray(b1, np.float32)
    W2 = np.asarray(W2, np.float32); b2 = np.asarray(b2, np.float32)
    Wl = np.asarray(Wl, np.float32); bl = np.asarray(bl, np.float32)

    counts, perm, caps = _plan(ind)
    g = _geometry(caps)

    key = tuple(caps)
    if key not in _cache:
        _cache[key] = _build_program(caps)
    nc = _cache[key]

    in_maps, rows = _prep_inputs(x, ind, W1, b1, W2, b2, Wl, bl, perm, caps, g)
    res = run_bass_kernel_spmd(nc, in_maps, core_ids=list(range(NCORES)))
    return _unscatter(res.results, rows, perm, caps, g)


# revision 62
# speedup vs baseline: 1.0208x; 1.0208x over previous
"""MoE routing kernel (nn_JSMLP): per-row expert-indexed 3-layer MLP.

  out[n] = Wl[i] @ tanh(W2[i] @ tanh(W1[i] @ x[n] + b1[i]) + b2[i]) + bl[i],  i = ind[n]

Sharding (hardcoded): expert-parallel across 8 cores, load-balanced by
count-sorted round-robin so a single SPMD program fits all cores:
experts are sorted by row count (desc); rank r goes to core r%8, slot r//8.
Slot s then has the same capacity cap[s] = roundup(count of rank 8s, 2) on
every core, so per-slot column spans are compile-time constants while padding
stays ~2% (vs ~50% for a uniform max-count capacity). Slots are grouped into
blocks of <=5 (span <=512 = one PSUM bank), executed in a pyramid order
(small block first for pipeline fill + PE p-state warm-up, big blocks
mid-stream, one 2-slot block last so the drain after the final weight
DMA is short).

Numerics: W1 (with b1 row), W2 and the first half of Wl are stored as
float8_e3m4 scaled by 64 (values land in e3m4's normal range; 4 mantissa
bits ~ 1.2% rms/elem); the 1/64 descale rides the tanh activations and the
output copy (tensor_scalar_mul) for free. Wl's second half is bf16 (also
x64, exact). x, h, biases stay bf16; PSUM accumulates fp32. Measured
end-to-end rel err 1.85e-2 vs the fp32 reference (gate: 2e-2).

Per core, per block b:
  L1: H1T[256, S_b] = W1augT.T @ [x;1]T   (bias via ones-row, K=65, e3m4 x64)
  L2: H2T[256, S_b] = W2T.T @ tanh(H1T/64)  (K=256 in 2 chunks; b2 added last
      via a tiny block-diag ones matmul in bf16, also x64)
  L3: outT[2x64, PS_b] = WlT.T @ tanh(H2T/64)  (2 experts per 128-partition
      tile; bl added last via pair-diag ones; plain bf16)
Within each PSUM accumulation group only the first matmul has start=True
(start marks the whole 2KB zero-region pending-zero, so per-region starts
would wipe neighbors); bias matmuls go last because the first matmul after a
pipeline stall runs at the mid p-state and should be a narrow chunk, not the
full-span bias. The block loop is software-pipelined as [L2(i), L3(i-1),
L1(i+2)] per iteration with one shared L1/L2 PSUM tile per block (3-deep
ring + two 1-bank L3 tiles = all 8 banks) so the in-order PE queue never
stalls on ACT between layers.

DMA plan (the shared DMA engine pool is the roofline at ~360 B/ns, and the
serial HWDGE costs ~630ns per DMA): x and W1 are packed per-block into one
byte-merged e3m4 tensor loaded in 3 slices; W2+Wl are packed per-block into
another (Wl columns are read through bitcast(bf16) APs), W2 loading ahead of
Wl for the first blocks and as one slab later; per-2-block stores ride the
idle GPSIMD/SWDGE path (no HWDGE slot), with the final store on SP. All
loads land in resident SBUF tiles (no ring reuse) so the load queue never
blocks on consumers.
"""

import numpy as np
import ml_dtypes

N, IN_DIM, H1, H2, LIN, NEXP = 16384, 64, 256, 256, 64, 256
NCORES = 8
SLOTS = NEXP // NCORES  # 32 experts per core

BF16 = ml_dtypes.bfloat16
E3M4 = ml_dtypes.float8_e3m4
WSCALE = 64.0

_cache = {}


def _geometry(caps):
    """Block/pair geometry shared by program builder and host prep.

    caps: per-slot capacities (len 32, multiples of 4, may be 0).
    Returns dict with blocks (list of slot-index lists), per-block slot
    offsets, block x-offsets, pair layout and output offsets.
    """
    blocks = []
    cur, cur_sum = [], 0
    for s in range(SLOTS):
        c = caps[s]
        if c == 0:
            continue
        if len(cur) == 6 or (cur_sum + c > 512 and cur):
            blocks.append(cur)
            cur, cur_sum = [], 0
        cur.append(s)
        cur_sum += c
    if cur:
        blocks.append(cur)
    # one tiny 2-slot trailing block shortens the drain after the last
    # weight DMA with a single end-chain round of engine hops
    if len(blocks[-1]) > 2:
        s2 = blocks[-1].pop()
        s1 = blocks[-1].pop()
        blocks.append([s1, s2])
    # pyramid execution order: a small block first (quick pipeline fill and
    # PE p-state warm-up), big blocks mid-stream, tiny block last (short
    # drain after the final weight DMA)
    if len(blocks) > 3:
        tail, normals = blocks[-1:], blocks[:-1]
        ns = sorted(normals, key=lambda bl: sum(caps[s] for s in bl))
        blocks = ns[0::2] + ns[1::2][::-1] + tail

    # equalize capacities within each L3 pair (~1.5% extra padding) so the
    # packed output tile has no gap columns — every PSUM element is written
    # by a chunk matmul before the trailing bias accumulation
    caps = list(caps)
    for bl in blocks:
        for i in range(0, len(bl) - 1, 2):
            m = max(caps[bl[i]], caps[bl[i + 1]])
            caps[bl[i]] = caps[bl[i + 1]] = m

    g = {"blocks": blocks, "caps": caps, "xoff": [], "S": [], "XO": [],
         "pairs": [], "poff": [], "PS": [], "OO": []}
    xo_total, oo_total = 0, 0
    for bl in blocks:
        offs, acc = [], 0
        for s in bl:
            offs.append(acc)
            acc += caps[s]
        assert acc <= 512, f"block span {acc} exceeds a PSUM bank"
        g["xoff"].append(offs)
        g["S"].append(acc)
        g["XO"].append(xo_total)
        xo_total += acc
        prs = [(bl[i], bl[i + 1] if i + 1 < len(bl) else None)
               for i in range(0, len(bl), 2)]
        poffs, pacc = [], 0
        for a, b in prs:
            poffs.append(pacc)
            pacc += max(caps[a], caps[b] if b is not None else 0)
        g["pairs"].append(prs)
        g["poff"].append(poffs)
        g["PS"].append(pacc)
        g["OO"].append(oo_total)
        oo_total += pacc
    g["TOT"] = xo_total
    g["TOT2"] = oo_total
    # weight tensors are laid out in execution order: slot s lives at
    # column-group scol[s], so per-block loads are contiguous slices
    scol, idx = {}, 0
    for bl in blocks:
        for s in bl:
            scol[s] = idx
            idx += 1
    g["scol"] = scol
    return g


def _build_program(caps):
    import concourse.bass as bass
    import concourse.tile as tile
    from concourse import bacc, mybir

    g = _geometry(caps)
    caps = g["caps"]
    blocks, S, XO, xoff = g["blocks"], g["S"], g["XO"], g["xoff"]
    pairs, poff, PS, OO = g["pairs"], g["poff"], g["PS"], g["OO"]
    NB = len(blocks)
    TOT, TOT2 = g["TOT"], g["TOT2"]
    scol = g["scol"]

    f32 = mybir.dt.float32
    bf16 = mybir.dt.bfloat16
    e3 = mybir.dt.float8e3
    Tanh = mybir.ActivationFunctionType.Tanh

    # ct columns: [w2c: NB*256 | wlc: NB*128 (rows 0:2) | bdt: TOT | bdl: TOT2]
    O_WLC = NB * 256
    O_BDT = O_WLC + NB * 128
    O_BDL = O_BDT + TOT
    CTW = O_BDL + TOT2

    nc = bacc.Bacc("TRN2", target_bir_lowering=False, debug=False,
                   num_devices=NCORES)

    # xw: per exec block [xg bytes (2*S_b) | w1 (nslots*256)], all e3m4 bytes
    XB = []
    off = 0
    for b in range(NB):
        XB.append(off)
        off += 2 * S[b] + 256 * len(blocks[b])
    XWW = off
    xw_d = nc.dram_tensor("xw", [65, XWW], e3, kind="ExternalInput")
    # wb: per exec block [w2: nslots*512 | wl(bf16 bytes): nslots*128],
    # so a block's W2 can load separately from (ahead of) its Wl
    WBO = []
    wboff = 0
    for b in range(NB):
        WBO.append(wboff)
        wboff += 704 * len(blocks[b])
    WBW = wboff
    wb_d = nc.dram_tensor("wb", [128, WBW], e3, kind="ExternalInput")
    ct_d = nc.dram_tensor("ct", [8, CTW], bf16, kind="ExternalInput")
    out_d = nc.dram_tensor("out", [128, TOT2], bf16, kind="ExternalOutput")

    # block -> first/last weight-column groups (exec-order layout)
    def slot_range(b0, b1):
        lo = scol[blocks[b0][0]]
        hi = scol[blocks[b1][-1]] + 1
        return lo, hi

    with tile.TileContext(nc) as tc:
        with (
            tc.tile_pool(name="stat", bufs=1) as spool,
            tc.tile_pool(name="acts", bufs=NB) as hpool,
            tc.tile_pool(name="ph", bufs=3, space=bass.MemorySpace.PSUM) as php,
            tc.tile_pool(name="po", bufs=2, space=bass.MemorySpace.PSUM) as pop,
        ):
            ct = spool.tile([8, CTW], bf16, tag="ct")
            xwt = spool.tile([65, XWW], e3, tag="xw")
            wbt = spool.tile([128, WBW], e3, tag="wb")
            ostage = spool.tile([128, TOT2], bf16, tag="out")

            # ---- load schedule -------------------------------------------
            # xw (x + W1, block-interleaved) in two slices; W2 per block in
            # execution order. Early blocks' Wl defers behind their W2 (L3
            # lags L2 by a block), later blocks load as one merged slab.
            def loadslice(lo, hi):
                nc.sync.dma_start(wbt[:, lo:hi], wb_d.ap()[:, lo:hi])

            def w2_rng(b):
                return WBO[b], WBO[b] + 512 * len(blocks[b])

            def wl_rng(b):
                return WBO[b] + 512 * len(blocks[b]), \
                    WBO[b] + 704 * len(blocks[b])

            NSPLIT = min(2, NB)
            b3 = min(2, NB - 1)
            b6 = min(5, NB - 1)
            xs1 = XB[b3] if 0 < b3 < NB else XWW
            xs2 = XB[b6] if b3 < b6 < NB else XWW
            nc.sync.dma_start(xwt[:, 0:xs1], xw_d.ap()[:, 0:xs1])
            loadslice(*w2_rng(0))
            nc.sync.dma_start(ct[:], ct_d.ap())
            if NB > 1:
                loadslice(*w2_rng(1))
            if xs1 < xs2:
                nc.sync.dma_start(xwt[:, xs1:xs2], xw_d.ap()[:, xs1:xs2])
            loadslice(*wl_rng(0))
            for b in range(2, NSPLIT):
                loadslice(*w2_rng(b))
                loadslice(*wl_rng(b - 1))
            if xs2 < XWW:
                nc.sync.dma_start(xwt[:, xs2:], xw_d.ap()[:, xs2:])
            if NSPLIT >= 2:
                loadslice(*wl_rng(NSPLIT - 1))
            for b in range(NSPLIT, NB):
                loadslice(WBO[b], WBO[b] + 704 * len(blocks[b]))

            # store after every 2nd block (idle Pool/SWDGE path, no HWDGE
            # slot); the final store rides SP whose HWDGE is free by then and
            # ~400ns faster than SWDGE on the drain-critical path
            store_after = {}
            lo_b = 0
            for b in range(1, NB, 2):
                store_after[b] = (lo_b, b)
                lo_b = b + 1
            if lo_b < NB:
                store_after[NB - 1] = (lo_b, NB - 1)

            def pool_activation(out, in_, func, scale=1.0):
                # InstActivation on the (otherwise idle) GPSIMD engine:
                # same semantics as nc.scalar.activation, modeled by the
                # cost model at the generic GPSIMD software efficiency
                eng = nc.gpsimd
                bias = eng.bass.const_aps.scalar_like(0.0, in_)
                ins = [eng.lower_ap(in_), eng.lower_ap(bias)]
                for arg in (scale, 0.0):
                    ins.append(mybir.ImmediateValue(dtype=f32, value=arg))
                return eng.add_instruction(mybir.InstActivation(
                    name=eng.bass.get_next_instruction_name(),
                    func=func, ins=ins, outs=[eng.lower_ap(out)]))

            # ---- software-pipelined compute ------------------------------
            # One PSUM tile per block serves both L1 and L2 (L2's start=True
            # rezeroes it after tanh1 has read it), so a 3-deep ring plus two
            # 1-bank L3 tiles fits the 8 PSUM banks without stalling.
            def emit_l1(b):
                bslots, Sb, xob = blocks[b], S[b], xoff[b]
                ph1 = php.tile([128, 1024], f32, tag="ph", name=f"ph_{b}")
                for t in range(2):
                    for i, s in enumerate(bslots):
                        c = caps[s]
                        w1o = XB[b] + 2 * Sb + i * 256 + t * 128
                        nc.tensor.matmul(
                            ph1[:, t * 512 + xob[i]: t * 512 + xob[i] + c],
                            xwt[:, w1o: w1o + 128],
                            xwt[:, XB[b] + 2 * xob[i]: XB[b] + 2 * (xob[i] + c)].bitcast(bf16),
                        )
                h1 = hpool.tile([128, 2 * Sb], bf16, tag="h1",
                                padded_shape=[128, 2 * max(S)], name=f"h1_{b}")
                nc.scalar.activation(
                    h1[:].rearrange("p (t s) -> p t s", t=2),
                    ph1[:].rearrange("p (t s) -> p t s", t=2)[:, :, 0:Sb],
                    Tanh, scale=1.0 / WSCALE,
                )
                return h1, ph1

            def emit_l2(b, h1, ph2):
                # bias matmul last: the first matmul after a pipeline stall
                # runs at the mid p-state, so it should be a narrow chunk
                # (F=c), not the full-span bias (F=S)
                bslots, Sb, xob = blocks[b], S[b], xoff[b]
                for t in range(2):
                    for i, s in enumerate(bslots):
                        c = caps[s]
                        nc.tensor.matmul(
                            ph2[:, t * 512 + xob[i]: t * 512 + xob[i] + c],
                            wbt[:, WBO[b] + i * 512 + t * 128: WBO[b] + i * 512 + (t + 1) * 128],
                            h1[:, xob[i]: xob[i] + c],
                            start=i == 0, stop=False, skip_group_check=True,
                        )
                        nc.tensor.matmul(
                            ph2[:, t * 512 + xob[i]: t * 512 + xob[i] + c],
                            wbt[:, WBO[b] + i * 512 + 256 + t * 128: WBO[b] + i * 512 + 256 + (t + 1) * 128],
                            h1[:, Sb + xob[i]: Sb + xob[i] + c],
                            start=False, stop=False, skip_group_check=True,
                        )
                    nc.tensor.matmul(
                        ph2[:, t * 512: t * 512 + Sb],
                        ct[:, (b * 2 + t) * 128: (b * 2 + t + 1) * 128],
                        ct[:, O_BDT + XO[b]: O_BDT + XO[b] + Sb],
                        start=False, stop=True, skip_group_check=True,
                    )
                h2 = hpool.tile([128, 2 * Sb], bf16, tag="h2",
                                padded_shape=[128, 2 * max(S)], name=f"h2_{b}")
                nc.scalar.activation(
                    h2[:].rearrange("p (t s) -> p t s", t=2),
                    ph2[:].rearrange("p (t s) -> p t s", t=2)[:, :, 0:Sb],
                    Tanh, scale=1.0 / WSCALE,
                )
                return h2

            def emit_l3(b, h2):
                Sb, xob = S[b], xoff[b]
                po = pop.tile([128, PS[b]], f32, tag="po",
                              padded_shape=[128, max(PS)], name=f"po_{b}")
                started = {0: False, 1: False}
                for ci, (sa, sb_) in enumerate(pairs[b]):
                    for h, s in enumerate((sa, sb_)):
                        if s is None:
                            continue
                        c = caps[s]
                        xo = xob[2 * ci + h]
                        wlo = WBO[b] + 512 * len(blocks[b]) + (2 * ci + h) * 192
                        nc.tensor.matmul(
                            po[h * 64:(h + 1) * 64,
                               poff[b][ci]: poff[b][ci] + c],
                            wbt[:, wlo: wlo + 64],
                            h2[:, xo: xo + c],
                            start=not started[h], stop=False,
                            skip_group_check=True,
                        )
                        started[h] = True
                        nc.tensor.matmul(
                            po[h * 64:(h + 1) * 64,
                               poff[b][ci]: poff[b][ci] + c],
                            wbt[:, wlo + 64: wlo + 192].bitcast(bf16),
                            h2[:, Sb + xo: Sb + xo + c],
                            start=False, stop=False, skip_group_check=True,
                        )
                # pair-gap columns may accumulate onto stale PSUM here; those
                # columns are never unscattered
                for h in range(2):
                    nc.tensor.matmul(
                        po[h * 64:(h + 1) * 64, 0:PS[b]],
                        ct[0:4, O_WLC + b * 128 + h * 64: O_WLC + b * 128 + (h + 1) * 64],
                        ct[0:4, O_BDL + OO[b]: O_BDL + OO[b] + PS[b]],
                        start=False, stop=h == 1, skip_group_check=True,
                    )
                nc.vector.tensor_scalar_mul(ostage[:, OO[b]: OO[b] + PS[b]],
                                            po[:, 0:PS[b]], 1.0 / WSCALE)
                if b in store_after:
                    b0, b1 = store_after[b]
                    eng = nc.sync if b1 == NB - 1 else nc.gpsimd
                    eng.dma_start(
                        out_d.ap()[:, OO[b0]: OO[b1] + PS[b1]],
                        ostage[:, OO[b0]: OO[b1] + PS[b1]],
                    )


            # software pipeline, PE order per iteration: L2(i), L3(i-1),
            # L1(i+2). The lookahead keeps every PSUM-ring WAR dependency one
            # full iteration old by the time it's needed, so the in-order PE
            # queue never stalls on ACT.
            h1s, h2s, phs = {}, {}, {}
            LOOK = min(2, NB)
            for i in range(LOOK):
                h1s[i], phs[i] = emit_l1(i)
            for i in range(NB):
                h2s[i] = emit_l2(i, h1s.pop(i), phs.pop(i))
                if i >= 1:
                    emit_l3(i - 1, h2s.pop(i - 1))
                if i + LOOK < NB:
                    h1s[i + LOOK], phs[i + LOOK] = emit_l1(i + LOOK)
            emit_l3(NB - 1, h2s.pop(NB - 1))

    nc.compile()
    return nc


def _plan(ind):
    counts = np.bincount(ind, minlength=NEXP)
    perm = np.argsort(-counts, kind="stable")
    caps = []
    for s in range(SLOTS):
        c = int(counts[perm[8 * s]])
        caps.append(0 if c == 0 else int(np.ceil(c / 2)) * 2)
    return counts, perm, caps


def _prep_inputs(x, ind, W1, b1, W2, b2, Wl, bl, perm, caps, g):
    """Build per-core arrays for the count-sorted round-robin layout."""
    blocks, S, XO, xoff = g["blocks"], g["S"], g["XO"], g["xoff"]
    pairs, poff, OO = g["pairs"], g["poff"], g["OO"]
    scol = g["scol"]
    caps = g["caps"]
    NB = len(blocks)
    TOT, TOT2 = g["TOT"], g["TOT2"]
    O_WLC = NB * 256
    O_BDT = O_WLC + NB * 128
    O_BDL = O_BDT + TOT
    CTW = O_BDL + TOT2

    order = np.argsort(ind, kind="stable")
    offs = np.zeros(NEXP + 1, np.int64)
    np.cumsum(np.bincount(ind, minlength=NEXP), out=offs[1:])
    rows = [order[offs[e]:offs[e + 1]] for e in range(NEXP)]

    # scaled transposed weights, shared across cores
    w1aug = np.concatenate([W1, b1[:, :, None]], axis=2)       # [E, 256, 65]
    w1q = (w1aug * WSCALE).astype(E3M4)                        # e3m4 x64
    w2q = (W2 * WSCALE).astype(E3M4)                           # [E, 256, 256]
    wlaq = (Wl[:, :, 0:128] * WSCALE).astype(E3M4)             # [E, 64, 128]
    wlbq = (Wl[:, :, 128:256] * WSCALE).astype(BF16)           # exact pow2
    b2q = (b2 * WSCALE).astype(np.float32)
    xb = x.astype(BF16)

    # xw/wb section offsets, mirroring the program builder
    XB, WBO = [], []
    off, wboff = 0, 0
    for b in range(NB):
        XB.append(off)
        off += 2 * S[b] + 256 * len(blocks[b])
        WBO.append(wboff)
        wboff += 704 * len(blocks[b])
    XWW, WBW = off, wboff

    in_maps = []
    for k in range(NCORES):
        xw = np.zeros((65, XWW), np.uint8)
        wb = np.zeros((128, WBW), np.uint8)
        ct = np.zeros((8, CTW), np.float32)
        ones = np.ones(1, BF16).view(np.uint8)
        for b in range(NB):
            Sb = S[b]
            for i, s in enumerate(blocks[b]):
                if caps[s] == 0:
                    continue
                e = perm[8 * s + k]
                r = rows[e]
                xo = XB[b] + 2 * xoff[b][i]
                xw[0:64, xo: xo + 2 * len(r)] = \
                    np.ascontiguousarray(xb[r].T).view(np.uint8)
                xw[64, xo: xo + 2 * caps[s]] = np.tile(ones, caps[s])
                w1o = XB[b] + 2 * Sb + i * 256
                xw[:, w1o: w1o + 256] = \
                    np.ascontiguousarray(w1q[e].T).view(np.uint8)
                w2o = WBO[b] + i * 512
                wb[:, w2o: w2o + 256] = \
                    np.ascontiguousarray(w2q[e, :, 0:128].T).view(np.uint8)
                wb[:, w2o + 256: w2o + 512] = \
                    np.ascontiguousarray(w2q[e, :, 128:256].T).view(np.uint8)
                wlo = WBO[b] + 512 * len(blocks[b]) + i * 192
                wb[:, wlo: wlo + 64] = \
                    np.ascontiguousarray(wlaq[e].T).view(np.uint8)
                wb[:, wlo + 64: wlo + 192] = \
                    np.ascontiguousarray(wlbq[e].T).view(np.uint8)
                ct[i, b * 256:(b + 1) * 256] = b2q[e]
                ct[i, O_BDT + XO[b] + xoff[b][i]:
                    O_BDT + XO[b] + xoff[b][i] + caps[s]] = 1.0
            for ci, (sa, sb_) in enumerate(pairs[b]):
                pc = OO[b] + poff[b][ci]
                w = max(caps[sa], caps[sb_] if sb_ is not None else 0)
                ct[ci, O_BDL + pc: O_BDL + pc + w] = 1.0
                for h, s in enumerate((sa, sb_)):
                    if s is None or caps[s] == 0:
                        continue
                    e = perm[8 * s + k]
                    ct[ci, O_WLC + b * 128 + h * 64: O_WLC + b * 128 + (h + 1) * 64] = bl[e] * WSCALE
        in_maps.append({
            "xw": xw.view(E3M4),
            "wb": wb.view(E3M4),
            "ct": ct.astype(BF16),
        })
    return in_maps, rows


def _unscatter(results, rows, perm, caps, g):
    blocks, xoff, poff, OO, pairs = g["blocks"], g["xoff"], g["poff"], g["OO"], g["pairs"]
    out = np.empty((N, LIN), np.float32)
    for k in range(NCORES):
        arr = np.asarray(results[k]["out"], np.float32)
        for b in range(len(blocks)):
            for ci, (sa, sb_) in enumerate(pairs[b]):
                for h, s in enumerate((sa, sb_)):
                    if s is None or caps[s] == 0:
                        continue
                    e = perm[8 * s + k]
                    r = rows[e]
                    col = OO[b] + poff[b][ci]
                    out[r, :] = arr[h * 64:(h + 1) * 64, col: col + len(r)].T
    return out


def kernel(x, ind, W1, b1, W2, b2, Wl, bl):
    from concourse.bass_utils import run_bass_kernel_spmd

    x = np.asarray(x, np.float32)
    ind = np.asarray(ind).astype(np.int64)
    W1 = np.asarray(W1, np.float32); b1 = np.asarray(b1, np.float32)
    W2 = np.asarray(W2, np.float32); b2 = np.asarray(b2, np.float32)
    Wl = np.asarray(Wl, np.float32); bl = np.asarray(bl, np.float32)

    counts, perm, caps = _plan(ind)
    g = _geometry(caps)

    key = tuple(caps)
    if key not in _cache:
        _cache[key] = _build_program(caps)
    nc = _cache[key]

    in_maps, rows = _prep_inputs(x, ind, W1, b1, W2, b2, Wl, bl, perm, caps, g)
    res = run_bass_kernel_spmd(nc, in_maps, core_ids=list(range(NCORES)))
    return _unscatter(res.results, rows, perm, caps, g)
